# revision 10
# baseline (speedup 1.0000x reference)
"""Trainium2 Bass kernel for a 24-layer Qwen2-style decoder with a custom
(token-type dependent) attention mask.

Sharding: sequence-parallel. 8 cores = (batch b in {0,1}) x (4 quarters of
the 2048-token sequence). Each core owns T=512 query tokens end-to-end
(norms, QKV, attention over the full 2048 keys, MLP, residual). The only
cross-core communication is a per-layer AllGather of the RoPE'd K and V
(bf16, split into two collectives so scores can start on K's arrival).

On-device layout is feature-major ("transposed activations"): activations
live as [feature, token] so every matmul's output directly feeds the next
matmul's moving operand. Per-token reductions (RMS-norm sum-of-squares,
softmax denominator) are done on the TensorEngine via ones-column matmuls /
an appended ones-column on V.

RoPE is folded into the projections: q_rope = (h@wq)*cos + (h@(wq@R^T))*sin
with R the rotate-half matrix, so the cross-partition rotation becomes a
second matmul plus two elementwise multiplies.

The mask is precomputed on the host as a multiplicative {0,1} mask applied
to exp(scores) (exact: exp(s + min_float) == 0 == exp(s) * 0).

Attention inner loop: scores for two key-chunks land in one 2-bank PSUM
tile so a single Exp activation and a single mask multiply cover both.
V is staged per kv-group in a 128-wide block whose layout places each
half's output (and its ones-column denominator row) directly on the
partitions where o_proj consumes it. Denominators for all 14 heads are
gathered into one PSUM tile via one-hot matmuls, inverted once with the
fast reciprocal, and broadcast back per chunk with selector matmuls.
"""

import sys

for _p in ("/opt/trn_rl_repo",):
    if _p not in sys.path:
        sys.path.insert(0, _p)

import numpy as np
import ml_dtypes

import concourse.bass as bass
import concourse.mybir as mybir
import concourse.tile as tile
from concourse import bacc
from concourse.bass_utils import run_bass_kernel_spmd
from concourse.masks import make_identity

F32 = mybir.dt.float32
BF16 = mybir.dt.bfloat16
AF = mybir.ActivationFunctionType

# model dims
D = 896
L = 24
HQ = 14
HKV = 2
DH = 64
I = 4864
EPS = 1e-6
THETA = 1e6
B = 2
S = 2048

# sharding
N_CORES = 8
G = 4                      # cores per batch group
T = S // G                 # 512 local query tokens per core
GROUPS = [[0, 1, 2, 3], [4, 5, 6, 7]]

KC = D // 128              # 7   K-chunks over hidden dim
MI = I // 128              # 38  M-chunks over intermediate dim
NKC = S // 128             # 16  chunks over key dim
NKP = NKC // 2             # 8   key-chunk pairs

N_LAYERS_OVERRIDE = None   # for testing with fewer layers

_BUILD_CACHE = {}
_LAST_IN_MAPS = None


def _build(n_layers, with_bias):
    nc = bacc.Bacc(num_devices=N_CORES)

    xT_p = nc.declare_dram_parameter("xT", [D, T], F32, isOutput=False)
    cos_p = nc.declare_dram_parameter("cosb", [128, T], F32, isOutput=False)
    sin_p = nc.declare_dram_parameter("sinb", [128, T], F32, isOutput=False)
    mask_p = nc.declare_dram_parameter("maskT", [S, T], BF16, isOutput=False)
    rotm_p = nc.declare_dram_parameter("rotm", [128, 128], BF16, isOutput=False)
    sel_p = nc.declare_dram_parameter("selp", [HQ, HQ + KC * 128], F32, isOutput=False)
    hot_p = nc.declare_dram_parameter("hotp", [128, HQ * HQ], F32, isOutput=False)
    wq_p = nc.declare_dram_parameter("wq", [n_layers, KC, KC, 128, 128], BF16, isOutput=False)
    wk_p = nc.declare_dram_parameter("wk", [n_layers, D, 128], BF16, isOutput=False)
    wv_p = nc.declare_dram_parameter("wv", [n_layers, D, 128], BF16, isOutput=False)
    wo_p = nc.declare_dram_parameter("wo", [n_layers, KC, KC, 128, 128], BF16, isOutput=False)
    wg_p = nc.declare_dram_parameter("wg", [n_layers, MI, KC, 128, 128], BF16, isOutput=False)
    wu_p = nc.declare_dram_parameter("wu", [n_layers, MI, KC, 128, 128], BF16, isOutput=False)
    wd_p = nc.declare_dram_parameter("wd", [n_layers, KC, MI, 128, 128], BF16, isOutput=False)
    if with_bias:
        bias_p = nc.declare_dram_parameter("biasp", [n_layers, 1152], BF16, isOutput=False)
    outT_p = nc.declare_dram_parameter("outT", [D, T], F32, isOutput=True)

    with tile.TileContext(nc) as tc:
        with tc.tile_pool(name="const", bufs=1) as constp, \
             tc.tile_pool(name="persist", bufs=1) as persist, \
             tc.tile_pool(name="wbig", bufs=2) as wbig, \
             tc.tile_pool(name="wsm", bufs=1) as wsm, \
             tc.tile_pool(name="wmlp", bufs=2) as wmlp, \
             tc.tile_pool(name="act", bufs=2) as actp, \
             tc.tile_pool(name="attn", bufs=1) as attnp, \
             tc.tile_pool(name="small", bufs=2) as smallp, \
             tc.tile_pool(name="expp", bufs=3) as expp, \
             tc.tile_pool(name="msbp", bufs=1) as msbp, \
             tc.tile_pool(name="ps", bufs=1, space="PSUM") as ps, \
             tc.tile_pool(name="dramp", bufs=2, space="DRAM") as dramp:

            ident = constp.tile([128, 128], BF16)
            make_identity(nc, ident)
            ones_col = constp.tile([128, 1], BF16)
            nc.vector.memset(ones_col, 1.0)
            ones_r128 = constp.tile([1, 128], F32)
            nc.vector.memset(ones_r128, 1.0)
            ones_r128b = constp.tile([1, 128], BF16)
            nc.vector.memset(ones_r128b, 1.0)
            ones_row = constp.tile([1, T], BF16)
            nc.vector.memset(ones_row, 1.0)
            eps_t = constp.tile([1, 1], F32)
            nc.vector.memset(eps_t, EPS)
            rotm_sb = constp.tile([128, 128], BF16)
            nc.sync.dma_start(out=rotm_sb, in_=rotm_p[:, :])
            sel_sb = constp.tile([HQ, HQ + KC * 128], F32)
            nc.sync.dma_start(out=sel_sb, in_=sel_p[:, :])
            hot_sb = constp.tile([128, HQ, HQ], F32)
            nc.sync.dma_start(out=hot_sb, in_=hot_p.rearrange("p (a b) -> p a b", a=HQ))

            xT_sb = persist.tile([128, KC, T], F32)
            nc.sync.dma_start(out=xT_sb, in_=xT_p.rearrange("(kc p) t -> p kc t", p=128))
            cos_sb = persist.tile([128, T], F32)
            nc.sync.dma_start(out=cos_sb, in_=cos_p[:, :])
            sin_sb = persist.tile([128, T], F32)
            nc.sync.dma_start(out=sin_sb, in_=sin_p[:, :])
            mask_sb = persist.tile([128, NKC, T], BF16)
            nc.sync.dma_start(out=mask_sb, in_=mask_p.rearrange("(kc p) t -> p kc t", p=128))

            # V staged per kv-group as a 128-wide block per key-chunk
            # (partitions = keys). Fixed columns (ones for the denominator
            # row, zeros for the unused half) are set once; the v data is
            # DMA'd in fresh each layer from the V AllGather.
            # half 0: cols 0:64 = v(group0), col 64 = ones, cols 65:128 = 0
            #   -> AV lands o on partitions 0:64, denominator on 64.
            # half 1: col 0 = ones, cols 1:64 = 0, cols 64:128 = v(group1)
            #   -> AV lands o on partitions 64:128, denominator on 0.
            vaug = persist.tile([128, NKC, 2, 128], BF16)
            nc.vector.memset(vaug, 0.0)
            nc.vector.memset(vaug[:, :, 0, 64:65], 1.0)
            nc.vector.memset(vaug[:, :, 1, 0:1], 1.0)

            def rmsnorm(out_dtype=BF16, tag="h"):
                ssq = ps.tile([1, T], F32, tag="w", bufs=2)
                for k in range(KC):
                    sq = smallp.tile([128, T], BF16, tag="sq")
                    nc.vector.tensor_mul(sq, xT_sb[:, k, :], xT_sb[:, k, :])
                    nc.tensor.matmul(ssq, ones_col, sq, start=(k == 0), stop=(k == KC - 1))
                rmsv = smallp.tile([1, T], F32, tag="rmsv")
                nc.scalar.activation(out=rmsv, in_=ssq, func=AF.Sqrt, bias=eps_t, scale=1.0 / D)
                rstd = smallp.tile([1, T], F32, tag="rstd")
                nc.vector.reciprocal_approx_fast(out=rstd, in_=rmsv)
                rb = ps.tile([128, T], F32, tag="aux", bufs=1)
                nc.tensor.matmul(rb, ones_r128, rstd, start=True, stop=True)
                h = actp.tile([128, KC, T], out_dtype, tag=tag)
                for k in range(KC):
                    nc.vector.tensor_mul(h[:, k, :], xT_sb[:, k, :], rb)
                return h

            for l in range(n_layers):
                # ---------------- attention ----------------
                h1 = rmsnorm()

                if with_bias:
                    bias_sb = wsm.tile([1, 2176], BF16, tag="bias")
                    nc.sync.dma_start(out=bias_sb, in_=bias_p[l, None, :])

                wk_sb = wsm.tile([128, KC, 128], BF16, tag="wk")
                nc.sync.dma_start(out=wk_sb, in_=wk_p[l].rearrange("(kc p) m -> p kc m", p=128))
                wv_sb = wsm.tile([128, KC, 128], BF16, tag="wv")
                nc.sync.dma_start(out=wv_sb, in_=wv_p[l].rearrange("(kc p) m -> p kc m", p=128))

                def proj_rope(wa_sb, boff_a, out_ap):
                    """out = rope(h1@wa + ba) = p*cos + (R@p)*sin, feature-major.
                    The rotate-half R is applied with one extra matmul against
                    the constant block-diagonal rotation matrix."""
                    pa = ps.tile([128, T], F32, tag="w", bufs=2)
                    for k in range(KC):
                        nc.tensor.matmul(pa, wa_sb[:, k, :], h1[:, k, :],
                                         start=(k == 0), stop=(not with_bias and k == KC - 1))
                    if with_bias:
                        nc.tensor.matmul(pa, bias_sb[:, boff_a:boff_a + 128], ones_row,
                                         start=False, stop=True)
                    qa_sb = smallp.tile([128, T], BF16, tag="ropec")
                    nc.scalar.copy(qa_sb, pa)
                    pb = ps.tile([128, T], F32, tag="w", bufs=2)
                    nc.tensor.matmul(pb, rotm_sb, qa_sb, start=True, stop=True)
                    ta = smallp.tile([128, T], BF16, tag="ropea")
                    nc.vector.tensor_mul(ta, pa, cos_sb)
                    tb = smallp.tile([128, T], BF16, tag="ropeb")
                    nc.vector.tensor_mul(tb, pb, sin_sb)
                    nc.vector.tensor_add(out_ap, ta, tb)

                # local k (RoPE'd) and v first, so the allgathers overlap the
                # q projections below; K and V gathered separately so scores
                # can start as soon as K lands.
                kT_loc = attnp.tile([128, T], BF16, tag="kT_loc")
                proj_rope(wk_sb, 896, kT_loc[:, :])
                k_in = dramp.tile([128, T], BF16, tag="k_in")
                nc.sync.dma_start(out=k_in, in_=kT_loc[:, :])
                k_out = dramp.tile([G, 128, T], BF16, tag="k_out")
                nc.gpsimd.collective_compute(
                    "AllGather", mybir.AluOpType.bypass,
                    replica_groups=GROUPS,
                    ins=[k_in.opt()], outs=[k_out.opt()],
                )

                # v projection in token-major layout (partitions = tokens) so
                # the gathered V drops straight into vaug via DMA — no
                # transposes on the PE and no staging copies.
                v_nat = attnp.tile([128, T // 128, 128], BF16, tag="v_nat")
                for tcq in range(T // 128):
                    pvn = ps.tile([128, 128], F32, tag="w", bufs=2)
                    for k in range(KC):
                        nc.tensor.matmul(pvn, h1[:, k, tcq * 128:(tcq + 1) * 128],
                                         wv_sb[:, k, :],
                                         start=(k == 0), stop=(not with_bias and k == KC - 1))
                    if with_bias:
                        nc.tensor.matmul(pvn, ones_r128b, bias_sb[:, 1024:1152],
                                         start=False, stop=True)
                    nc.vector.tensor_copy(v_nat[:, tcq, :], pvn)
                v_in = dramp.tile([128, T // 128, 128], BF16, tag="v_in")
                nc.sync.dma_start(out=v_in, in_=v_nat[:, :, :])
                v_out = dramp.tile([G, 128, T // 128, 128], BF16, tag="v_out")
                nc.gpsimd.collective_compute(
                    "AllGather", mybir.AluOpType.bypass,
                    replica_groups=GROUPS,
                    ins=[v_in.opt()], outs=[v_out.opt()],
                )

                # q projections (by M-chunk; chunk mc holds heads mc, mc+7)
                qT = attnp.tile([128, KC, T], BF16, tag="qT")
                for mc in range(KC):
                    wqc = wbig.tile([128, KC, 128], BF16, tag="wq")
                    nc.sync.dma_start(out=wqc, in_=wq_p[l, mc].rearrange("kc p m -> p kc m"))
                    proj_rope(wqc, mc * 128, qT[:, mc, :])

                kT_full = attnp.tile([128, S], BF16, tag="kT_full")
                for r in range(G):
                    nc.sync.dma_start(out=kT_full[:, r * T:(r + 1) * T], in_=k_out[r])
                NTC = T // 128
                for r in range(G):
                    nc.sync.dma_start(out=vaug[:, r * NTC:(r + 1) * NTC, 0, 0:64],
                                      in_=v_out[r, :, :, 0:64])
                    nc.sync.dma_start(out=vaug[:, r * NTC:(r + 1) * NTC, 1, 64:128],
                                      in_=v_out[r, :, :, 64:128])

                # attention per head. Host permutes q-head order so chunk mc
                # holds heads (mc, mc+7): head h sits at partition base
                # (h//7)*64 == its kv-group's base in kT_full. Two key-chunks
                # share one 2-bank PSUM tile -> one Exp + one mask-mul per
                # pair. Denominators of all heads are collected into one PSUM
                # tile by one-hot matmuls (f32), inverted once, and broadcast
                # per chunk with selector matmuls.
                oT = attnp.tile([128, KC, T], BF16, tag="oT")
                aux = ps.tile([128, T], F32, tag="aux", bufs=1)
                for h in range(HQ):
                    mc, half = h % KC, h // KC
                    g = half
                    oacc = ps.tile([128, T], F32, tag="oacc", bufs=3)
                    for kp in range(NKP):
                        sp = ps.tile([128, 2, T], F32, tag="w", bufs=2)
                        for j in (0, 1):
                            kc = 2 * kp + j
                            nc.tensor.matmul(
                                sp[:, j, :],
                                kT_full[g * 64:g * 64 + 64, kc * 128:(kc + 1) * 128],
                                qT[half * 64:half * 64 + 64, mc, :],
                                start=True, stop=True)
                        ex = expp.tile([128, 2, T], BF16, tag="expT")
                        nc.scalar.activation(out=ex, in_=sp, func=AF.Exp, scale=0.125)
                        nc.vector.tensor_mul(ex, ex, mask_sb[:, 2 * kp:2 * kp + 2, :])
                        for j in (0, 1):
                            kc = 2 * kp + j
                            nc.tensor.matmul(oacc, vaug[:, kc, half, :], ex[:, j, :],
                                             start=(kc == 0), stop=(kc == NKC - 1))
                    drow = 64 if half == 0 else 0
                    dsb = smallp.tile([128, T], F32, tag="dsb")
                    nc.scalar.copy(dsb[drow:drow + 1, :], oacc[drow:drow + 1, :])
                    nc.tensor.matmul(aux[0:HQ, :], hot_sb[drow:drow + 1, h, :],
                                     dsb[drow:drow + 1, :],
                                     start=(h == 0), stop=(h == HQ - 1))
                    if half == 0:
                        nc.vector.tensor_copy(oT[0:64, mc, :], oacc[0:64, :])
                    else:
                        nc.vector.tensor_copy(oT[64:128, mc, :], oacc[64:128, :])

                rec = smallp.tile([HQ, T], F32, tag="rec")
                nc.vector.reciprocal_approx_fast(out=rec, in_=aux[0:HQ, :])

                # normalize oT chunk-by-chunk, o_proj + residual right behind
                for mc in range(KC):
                    rb = ps.tile([128, T], F32, tag="aux", bufs=1)
                    nc.tensor.matmul(rb, sel_sb[:, HQ + mc * 128:HQ + (mc + 1) * 128],
                                     rec, start=True, stop=True)
                    nc.vector.tensor_mul(oT[:, mc, :], oT[:, mc, :], rb)
                for mc in range(KC):
                    woc = wbig.tile([128, KC, 128], BF16, tag="wo")
                    nc.sync.dma_start(out=woc, in_=wo_p[l, mc].rearrange("kc p m -> p kc m"))
                    xd = ps.tile([128, T], F32, tag="w", bufs=2)
                    for k in range(KC):
                        nc.tensor.matmul(xd, woc[:, k, :], oT[:, k, :],
                                         start=(k == 0), stop=(k == KC - 1))
                    nc.vector.tensor_add(xT_sb[:, mc, :], xT_sb[:, mc, :], xd)

                # ---------------- MLP ----------------
                h2 = rmsnorm()
                m_sb = msbp.tile([128, MI, T], BF16, tag="m")
                for mi in range(MI):
                    wgc = wmlp.tile([128, KC, 128], BF16, tag="wg")
                    nc.sync.dma_start(out=wgc, in_=wg_p[l, mi].rearrange("kc p m -> p kc m"))
                    wuc = wmlp.tile([128, KC, 128], BF16, tag="wu")
                    nc.sync.dma_start(out=wuc, in_=wu_p[l, mi].rearrange("kc p m -> p kc m"))
                    pg = ps.tile([128, T], F32, tag="w", bufs=2)
                    for k in range(KC):
                        nc.tensor.matmul(pg, wgc[:, k, :], h2[:, k, :],
                                         start=(k == 0), stop=(k == KC - 1))
                    pu = ps.tile([128, T], F32, tag="w", bufs=2)
                    for k in range(KC):
                        nc.tensor.matmul(pu, wuc[:, k, :], h2[:, k, :],
                                         start=(k == 0), stop=(k == KC - 1))
                    sg = smallp.tile([128, T], BF16, tag="sg")
                    nc.scalar.activation(out=sg, in_=pg, func=AF.Silu)
                    nc.vector.tensor_mul(m_sb[:, mi, :], sg, pu)
                for mc in range(KC):
                    wdc = wmlp.tile([128, MI, 128], BF16, tag="wd")
                    nc.sync.dma_start(out=wdc, in_=wd_p[l, mc].rearrange("kci p m -> p kci m"))
                    xd = ps.tile([128, T], F32, tag="w", bufs=2)
                    for ki in range(MI):
                        nc.tensor.matmul(xd, wdc[:, ki, :], m_sb[:, ki, :],
                                         start=(ki == 0), stop=(ki == MI - 1))
                    nc.vector.tensor_add(xT_sb[:, mc, :], xT_sb[:, mc, :], xd)

            # final norm (lnf applied on host)
            ssq = ps.tile([1, T], F32, tag="w", bufs=2)
            for k in range(KC):
                sq = smallp.tile([128, T], BF16, tag="sq")
                nc.vector.tensor_mul(sq, xT_sb[:, k, :], xT_sb[:, k, :])
                nc.tensor.matmul(ssq, ones_col, sq, start=(k == 0), stop=(k == KC - 1))
            rmsv = smallp.tile([1, T], F32, tag="rmsv")
            nc.scalar.activation(out=rmsv, in_=ssq, func=AF.Sqrt, bias=eps_t, scale=1.0 / D)
            rstd = smallp.tile([1, T], F32, tag="rstd")
            nc.vector.reciprocal_approx_fast(out=rstd, in_=rmsv)
            rb = ps.tile([128, T], F32, tag="aux", bufs=1)
            nc.tensor.matmul(rb, ones_r128, rstd, start=True, stop=True)
            outT_r = outT_p.rearrange("(kc p) t -> p kc t", p=128)
            for k in range(KC):
                oc = smallp.tile([128, T], F32, tag="outc")
                nc.vector.tensor_mul(oc, xT_sb[:, k, :], rb)
                nc.sync.dma_start(out=outT_r[:, k, :], in_=oc)

    nc.finalize()
    return nc


def get_kernel(n_layers, with_bias):
    key = (n_layers, with_bias)
    if key not in _BUILD_CACHE:
        _BUILD_CACHE[key] = _build(n_layers, with_bias)
    return _BUILD_CACHE[key]


def _bf(a):
    return np.asarray(a, dtype=np.float32).astype(ml_dtypes.bfloat16)


def _rot_cols(w):
    """w @ R^T per 64-wide head block: out[:, :32] = -w[:, 32:64], out[:, 32:] = w[:, :32]."""
    nh = w.shape[1] // DH
    wr = w.reshape(w.shape[0], nh, DH)
    out = np.empty_like(wr)
    out[:, :, :DH // 2] = -wr[:, :, DH // 2:]
    out[:, :, DH // 2:] = wr[:, :, :DH // 2]
    return out.reshape(w.shape)


def _rot_vec(b):
    nh = b.shape[-1] // DH
    br = b.reshape(-1, nh, DH)
    out = np.empty_like(br)
    out[:, :, :DH // 2] = -br[:, :, DH // 2:]
    out[:, :, DH // 2:] = br[:, :, :DH // 2]
    return out.reshape(b.shape)


def _pack_mk(w):
    """[Din, Dout] -> [mc, kc, p, m] chunks for lhsT streaming."""
    din, dout = w.shape
    return np.ascontiguousarray(
        w.reshape(din // 128, 128, dout // 128, 128).transpose(2, 0, 1, 3))


def prepare_in_maps(inputs, n_layers, with_bias):
    return _prepare(n_layers=n_layers, with_bias_override=with_bias, **inputs)[0]


def _prepare(inputs_embeds, token_type_ids, attention_mask,
             wq, bq, wk, bk, wv, bv, wo, wg, wu, wd, ln1, ln2, lnf,
             n_layers=None, with_bias_override=None):
    f32 = np.float32
    inputs_embeds = np.asarray(inputs_embeds, f32)
    token_type_ids = np.asarray(token_type_ids)
    attention_mask = np.asarray(attention_mask, f32)
    wq, bq, wk, bk = np.asarray(wq, f32), np.asarray(bq, f32), np.asarray(wk, f32), np.asarray(bk, f32)
    wv, bv, wo = np.asarray(wv, f32), np.asarray(bv, f32), np.asarray(wo, f32)
    wg, wu, wd = np.asarray(wg, f32), np.asarray(wu, f32), np.asarray(wd, f32)
    ln1, ln2, lnf = np.asarray(ln1, f32), np.asarray(ln2, f32), np.asarray(lnf, f32)

    if n_layers is None:
        n_layers = N_LAYERS_OVERRIDE if N_LAYERS_OVERRIDE is not None else L
    with_bias = bool(np.any(bq[:n_layers]) or np.any(bk[:n_layers]) or np.any(bv[:n_layers]))
    if with_bias_override is not None:
        with_bias = with_bias or with_bias_override

    # ---- per-layer weight packing (ln folded in; RoPE rotation folded in) ----
    # head permutation: q-chunk mc holds heads (mc, mc+7) so that each head's
    # partition half matches its GQA kv-group's rows in kT_full
    perm = [h for p in range(KC) for h in (p, p + KC)]
    inv_sl = np.array(perm)

    def _perm_qcols(w):                    # permute 64-wide head column blocks
        return np.ascontiguousarray(
            w.reshape(w.shape[0], HQ, DH)[:, inv_sl, :].reshape(w.shape[0], HQ * DH))

    def _perm_orows(w):                    # permute 64-wide head row blocks
        return np.ascontiguousarray(
            w.reshape(HQ, DH, w.shape[1])[inv_sl].reshape(HQ * DH, w.shape[1]))

    wq_eff = ln1[:, :, None] * wq          # [L, D, 896]
    wk_eff = ln1[:, :, None] * wk          # [L, D, 128]
    wv_eff = ln1[:, :, None] * wv
    wg_eff = ln2[:, :, None] * wg
    wu_eff = ln2[:, :, None] * wu

    wq_pack = np.stack([_pack_mk(_perm_qcols(wq_eff[l])) for l in range(n_layers)])
    wo_pack = np.stack([_pack_mk(_perm_orows(wo[l])) for l in range(n_layers)])
    wg_pack = np.stack([_pack_mk(wg_eff[l]) for l in range(n_layers)])
    wu_pack = np.stack([_pack_mk(wu_eff[l]) for l in range(n_layers)])
    wd_pack = np.stack([_pack_mk(wd[l]) for l in range(n_layers)])
    wk_arr = wk_eff[:n_layers]
    wv_arr = wv_eff[:n_layers]

    # block-diag rotate-half matrix (two 64-head blocks), as lhsT = R^T
    r64 = np.zeros((DH, DH), np.float32)
    r64[:DH // 2, DH // 2:] = -np.eye(DH // 2, dtype=np.float32)
    r64[DH // 2:, :DH // 2] = np.eye(DH // 2, dtype=np.float32)
    rot2 = np.zeros((128, 128), np.float32)
    rot2[:DH, :DH] = r64.T
    rot2[DH:, DH:] = r64.T

    # selector constants (f32): cols 0:HQ = identity(HQ) for the one-hot
    # denominator gather; cols HQ+mc*128 : HQ+(mc+1)*128 broadcast head mc
    # (partitions 0:64) and head mc+7 (partitions 64:128) for chunk mc.
    hotp = np.tile(np.eye(HQ, dtype=np.float32).reshape(1, HQ * HQ), (128, 1))
    selp = np.zeros((HQ, HQ + KC * 128), np.float32)
    selp[:, :HQ] = np.eye(HQ, dtype=np.float32)
    for mc in range(KC):
        selp[mc, HQ + mc * 128: HQ + mc * 128 + 64] = 1.0
        selp[mc + KC, HQ + mc * 128 + 64: HQ + (mc + 1) * 128] = 1.0

    base = {
        "wq": _bf(wq_pack), "wk": _bf(wk_arr), "wv": _bf(wv_arr),
        "wo": _bf(wo_pack), "wg": _bf(wg_pack), "wu": _bf(wu_pack), "wd": _bf(wd_pack),
        "rotm": _bf(rot2), "selp": selp, "hotp": hotp,
    }
    if with_bias:
        def _perm_b(b):
            return b.reshape(n_layers, HQ, DH)[:, inv_sl, :].reshape(n_layers, HQ * DH)
        bias_pack = np.concatenate(
            [_perm_b(bq[:n_layers]), bk[:n_layers], bv[:n_layers]], axis=1)
        base["biasp"] = _bf(bias_pack)

    # ---- RoPE tables ----
    inv_freq = 1.0 / (THETA ** (np.arange(0, DH, 2, dtype=f32) / DH))
    ang = np.arange(S, dtype=f32)[:, None] * inv_freq[None, :]      # [S, 32]
    emb = np.concatenate([ang, ang], axis=-1)                        # [S, DH]
    cos_full, sin_full = np.cos(emb), np.sin(emb)                    # [S, DH]

    # ---- mask (multiplicative, per batch) ----
    t = token_type_ids
    tq = t[:, :, None]
    tk = t[:, None, :]
    qi = np.arange(S)[:, None]
    ki = np.arange(S)[None, :]
    allowed = ((tq == 0) & (tk == 0)) | ((tq == 1) & ((tk == 0) | ((tk == 1) & (ki <= qi))))
    m = allowed.astype(f32) * (attention_mask[:, None, :] > 0.5)     # [B, S(q), S(k)]

    in_maps = []
    for c in range(N_CORES):
        b, qt = c // G, c % G
        q0 = qt * T
        im = dict(base)
        im["xT"] = np.ascontiguousarray(inputs_embeds[b, q0:q0 + T, :].T)
        im["cosb"] = np.ascontiguousarray(np.tile(cos_full[q0:q0 + T].T, (2, 1)).astype(f32))
        im["sinb"] = np.ascontiguousarray(np.tile(sin_full[q0:q0 + T].T, (2, 1)).astype(f32))
        im["maskT"] = _bf(np.ascontiguousarray(m[b, q0:q0 + T, :].T))
        in_maps.append(im)

    global _LAST_IN_MAPS
    _LAST_IN_MAPS = in_maps
    return in_maps, n_layers, with_bias


def kernel(**inputs):
    in_maps, n_layers, with_bias = _prepare(**inputs)
    nc = get_kernel(n_layers, with_bias)
    res = run_bass_kernel_spmd(nc, in_maps, list(range(N_CORES)))
    lnf = np.asarray(inputs["lnf"], np.float32)
    out = np.empty((B, S, D), dtype=np.float32)
    for c in range(N_CORES):
        b, qt = c // G, c % G
        out[b, qt * T:(qt + 1) * T, :] = res.results[c]["outT"].T
    out *= lnf[None, None, :]
    return out


# revision 11
# speedup vs baseline: 1.2681x; 1.2681x over previous
"""Trainium2 Bass kernel for a 24-layer Qwen2-style decoder with a custom
(token-type dependent) attention mask.

Sharding: sequence-parallel. 8 cores = (batch b in {0,1}) x (4 quarters of
the 2048-token sequence). Each core owns T=512 query tokens end-to-end
(norms, QKV, attention over the full 2048 keys, MLP, residual). The only
cross-core communication is a per-layer AllGather of the RoPE'd K and V
(bf16, split into two collectives so scores can start on K's arrival).

On-device layout is feature-major ("transposed activations"): activations
live as [feature, token] so every matmul's output directly feeds the next
matmul's moving operand. Per-token reductions (RMS-norm sum-of-squares,
softmax denominator) are done on the TensorEngine via ones-column matmuls /
an appended ones-column on V.

RoPE is folded into the projections: q_rope = (h@wq)*cos + (h@(wq@R^T))*sin
with R the rotate-half matrix, so the cross-partition rotation becomes a
second matmul plus two elementwise multiplies.

The mask is precomputed on the host as a multiplicative {0,1} mask applied
to exp(scores) (exact: exp(s + min_float) == 0 == exp(s) * 0).

Attention inner loop: scores for two key-chunks land in one 2-bank PSUM
tile so a single Exp activation and a single mask multiply cover both.
V is staged per kv-group in a 128-wide block whose layout places each
half's output (and its ones-column denominator row) directly on the
partitions where o_proj consumes it. Denominators for all 14 heads are
gathered into one PSUM tile via one-hot matmuls, inverted once with the
fast reciprocal, and broadcast back per chunk with selector matmuls.
"""

import sys

for _p in ("/opt/trn_rl_repo",):
    if _p not in sys.path:
        sys.path.insert(0, _p)

import numpy as np
import ml_dtypes

import concourse.bass as bass
import concourse.mybir as mybir
import concourse.tile as tile
from concourse import bacc
from concourse.bass_utils import run_bass_kernel_spmd
from concourse.masks import make_identity

F32 = mybir.dt.float32
BF16 = mybir.dt.bfloat16
AF = mybir.ActivationFunctionType

# model dims
D = 896
L = 24
HQ = 14
HKV = 2
DH = 64
I = 4864
EPS = 1e-6
THETA = 1e6
B = 2
S = 2048

# sharding
N_CORES = 8
G = 4                      # cores per batch group
T = S // G                 # 512 local query tokens per core
GROUPS = [[0, 1, 2, 3], [4, 5, 6, 7]]

KC = D // 128              # 7   K-chunks over hidden dim
MI = I // 128              # 38  M-chunks over intermediate dim
NKC = S // 128             # 16  chunks over key dim
NKP = NKC // 2             # 8   key-chunk pairs

N_LAYERS_OVERRIDE = None   # for testing with fewer layers

_BUILD_CACHE = {}
_LAST_IN_MAPS = None


def _build(n_layers, with_bias):
    nc = bacc.Bacc(num_devices=N_CORES)

    xT_p = nc.declare_dram_parameter("xT", [D, T], F32, isOutput=False)
    cos_p = nc.declare_dram_parameter("cosb", [128, T], F32, isOutput=False)
    sin_p = nc.declare_dram_parameter("sinb", [128, T], F32, isOutput=False)
    mask_p = nc.declare_dram_parameter("maskT", [S, T], BF16, isOutput=False)
    rotm_p = nc.declare_dram_parameter("rotm", [128, 128], BF16, isOutput=False)
    sel_p = nc.declare_dram_parameter("selp", [HQ, HQ + KC * 128], BF16, isOutput=False)
    hot_p = nc.declare_dram_parameter("hotp", [128, HQ * HQ], BF16, isOutput=False)
    wq_p = nc.declare_dram_parameter("wq", [n_layers, KC, KC, 128, 128], BF16, isOutput=False)
    wk_p = nc.declare_dram_parameter("wk", [n_layers, D, 128], BF16, isOutput=False)
    wv_p = nc.declare_dram_parameter("wv", [n_layers, D, 128], BF16, isOutput=False)
    wo_p = nc.declare_dram_parameter("wo", [n_layers, KC, KC, 128, 128], BF16, isOutput=False)
    wg_p = nc.declare_dram_parameter("wg", [n_layers, MI, KC, 128, 128], BF16, isOutput=False)
    wu_p = nc.declare_dram_parameter("wu", [n_layers, MI, KC, 128, 128], BF16, isOutput=False)
    wd_p = nc.declare_dram_parameter("wd", [n_layers, KC, MI, 128, 128], BF16, isOutput=False)
    if with_bias:
        bias_p = nc.declare_dram_parameter("biasp", [n_layers, 1152], BF16, isOutput=False)
    outT_p = nc.declare_dram_parameter("outT", [D, T], F32, isOutput=True)

    with tile.TileContext(nc) as tc:
        with tc.tile_pool(name="const", bufs=1) as constp, \
             tc.tile_pool(name="persist", bufs=1) as persist, \
             tc.tile_pool(name="wbig", bufs=2) as wbig, \
             tc.tile_pool(name="wsm", bufs=1) as wsm, \
             tc.tile_pool(name="wmlp", bufs=2) as wmlp, \
             tc.tile_pool(name="act", bufs=2) as actp, \
             tc.tile_pool(name="attn", bufs=1) as attnp, \
             tc.tile_pool(name="small", bufs=2) as smallp, \
             tc.tile_pool(name="expp", bufs=3) as expp, \
             tc.tile_pool(name="msbp", bufs=1) as msbp, \
             tc.tile_pool(name="ps", bufs=1, space="PSUM") as ps, \
             tc.tile_pool(name="dramp", bufs=2, space="DRAM") as dramp:

            ident = constp.tile([128, 128], BF16)
            make_identity(nc, ident)
            ones_col = constp.tile([128, 1], BF16)
            nc.vector.memset(ones_col, 1.0)
            ones_r128 = constp.tile([1, 128], F32)
            nc.vector.memset(ones_r128, 1.0)
            ones_r128b = constp.tile([1, 128], BF16)
            nc.vector.memset(ones_r128b, 1.0)
            ones_row = constp.tile([1, T], BF16)
            nc.vector.memset(ones_row, 1.0)
            eps_t = constp.tile([1, 1], F32)
            nc.vector.memset(eps_t, EPS)
            rotm_sb = constp.tile([128, 128], BF16)
            nc.sync.dma_start(out=rotm_sb, in_=rotm_p[:, :])
            sel_sb = constp.tile([HQ, HQ + KC * 128], BF16)
            nc.sync.dma_start(out=sel_sb, in_=sel_p[:, :])
            hot_sb = constp.tile([128, HQ, HQ], BF16)
            nc.sync.dma_start(out=hot_sb, in_=hot_p.rearrange("p (a b) -> p a b", a=HQ))

            xT_sb = persist.tile([128, KC, T], F32)
            nc.sync.dma_start(out=xT_sb, in_=xT_p.rearrange("(kc p) t -> p kc t", p=128))
            cos_sb = persist.tile([128, T], F32)
            nc.sync.dma_start(out=cos_sb, in_=cos_p[:, :])
            sin_sb = persist.tile([128, T], F32)
            nc.sync.dma_start(out=sin_sb, in_=sin_p[:, :])
            mask_sb = persist.tile([128, NKC, T], BF16)
            nc.sync.dma_start(out=mask_sb, in_=mask_p.rearrange("(kc p) t -> p kc t", p=128))

            # V staged per kv-group as a 128-wide block per key-chunk
            # (partitions = keys). Fixed columns (ones for the denominator
            # row, zeros for the unused half) are set once; the v data is
            # DMA'd in fresh each layer from the V AllGather.
            # half 0: cols 0:64 = v(group0), col 64 = ones, cols 65:128 = 0
            #   -> AV lands o on partitions 0:64, denominator on 64.
            # half 1: col 0 = ones, cols 1:64 = 0, cols 64:128 = v(group1)
            #   -> AV lands o on partitions 64:128, denominator on 0.
            vaug = persist.tile([128, NKC, 2, 128], BF16)
            nc.vector.memset(vaug, 0.0)
            nc.vector.memset(vaug[:, :, 0, 64:65], 1.0)
            nc.vector.memset(vaug[:, :, 1, 0:1], 1.0)

            # K for scores, zero-padded per kv-group to a full K=128
            # contraction: [:, 0, :] rows 0:64 = K(group0), rows 64:128 = 0;
            # [:, 1, :] rows 0:64 = 0, rows 64:128 = K(group1). The zero half
            # multiplies the other head's q rows and contributes nothing, so
            # the full 128-partition qT chunk is the moving operand.
            kT_pad = persist.tile([128, 2, S], BF16)
            nc.vector.memset(kT_pad, 0.0)

            def rmsnorm(out_dtype=BF16, tag="h"):
                ssq = ps.tile([1, T], F32, tag="w", bufs=2)
                for k in range(KC):
                    sq = smallp.tile([128, T], BF16, tag="sq")
                    nc.vector.tensor_mul(sq, xT_sb[:, k, :], xT_sb[:, k, :])
                    nc.tensor.matmul(ssq, ones_col, sq, start=(k == 0), stop=(k == KC - 1))
                rmsv = smallp.tile([1, T], F32, tag="rmsv")
                nc.scalar.activation(out=rmsv, in_=ssq, func=AF.Sqrt, bias=eps_t, scale=1.0 / D)
                rstd = smallp.tile([1, T], F32, tag="rstd")
                nc.vector.reciprocal_approx_fast(out=rstd, in_=rmsv)
                rb = ps.tile([128, T], F32, tag="aux", bufs=1)
                nc.tensor.matmul(rb, ones_r128, rstd, start=True, stop=True)
                h = actp.tile([128, KC, T], out_dtype, tag=tag)
                for k in range(KC):
                    nc.vector.tensor_mul(h[:, k, :], xT_sb[:, k, :], rb)
                return h

            for l in range(n_layers):
                # ---------------- attention ----------------
                h1 = rmsnorm()

                if with_bias:
                    bias_sb = wsm.tile([1, 2176], BF16, tag="bias")
                    nc.sync.dma_start(out=bias_sb, in_=bias_p[l, None, :])

                wk_sb = wsm.tile([128, KC, 128], BF16, tag="wk")
                nc.sync.dma_start(out=wk_sb, in_=wk_p[l].rearrange("(kc p) m -> p kc m", p=128))
                wv_sb = wsm.tile([128, KC, 128], BF16, tag="wv")
                nc.sync.dma_start(out=wv_sb, in_=wv_p[l].rearrange("(kc p) m -> p kc m", p=128))

                def proj_rope(wa_sb, boff_a, out_ap):
                    """out = rope(h1@wa + ba) = p*cos + (R@p)*sin, feature-major.
                    The rotate-half R is applied with one extra matmul against
                    the constant block-diagonal rotation matrix."""
                    pa = ps.tile([128, T], F32, tag="w", bufs=2)
                    for k in range(KC):
                        nc.tensor.matmul(pa, wa_sb[:, k, :], h1[:, k, :],
                                         start=(k == 0), stop=(not with_bias and k == KC - 1))
                    if with_bias:
                        nc.tensor.matmul(pa, bias_sb[:, boff_a:boff_a + 128], ones_row,
                                         start=False, stop=True)
                    qa_sb = smallp.tile([128, T], BF16, tag="ropec")
                    nc.scalar.copy(qa_sb, pa)
                    pb = ps.tile([128, T], F32, tag="w", bufs=2)
                    nc.tensor.matmul(pb, rotm_sb, qa_sb, start=True, stop=True)
                    ta = smallp.tile([128, T], BF16, tag="ropea")
                    nc.vector.tensor_mul(ta, pa, cos_sb)
                    tb = smallp.tile([128, T], BF16, tag="ropeb")
                    nc.vector.tensor_mul(tb, pb, sin_sb)
                    nc.vector.tensor_add(out_ap, ta, tb)

                # local k (RoPE'd) and v first, so the allgathers overlap the
                # q projections below; K and V gathered separately so scores
                # can start as soon as K lands.
                kT_loc = attnp.tile([128, T], BF16, tag="kT_loc")
                proj_rope(wk_sb, 896, kT_loc[:, :])
                k_in = dramp.tile([128, T], BF16, tag="k_in")
                nc.sync.dma_start(out=k_in, in_=kT_loc[:, :])
                k_out = dramp.tile([G, 128, T], BF16, tag="k_out")
                nc.gpsimd.collective_compute(
                    "AllGather", mybir.AluOpType.bypass,
                    replica_groups=GROUPS,
                    ins=[k_in.opt()], outs=[k_out.opt()],
                )

                # v projection in token-major layout (partitions = tokens) so
                # the gathered V drops straight into vaug via DMA — no
                # transposes on the PE and no staging copies.
                v_nat = attnp.tile([128, T // 128, 128], BF16, tag="v_nat")
                for tcq in range(T // 128):
                    pvn = ps.tile([128, 128], F32, tag="w", bufs=2)
                    for k in range(KC):
                        nc.tensor.matmul(pvn, h1[:, k, tcq * 128:(tcq + 1) * 128],
                                         wv_sb[:, k, :],
                                         start=(k == 0), stop=(not with_bias and k == KC - 1))
                    if with_bias:
                        nc.tensor.matmul(pvn, ones_r128b, bias_sb[:, 1024:1152],
                                         start=False, stop=True)
                    nc.vector.tensor_copy(v_nat[:, tcq, :], pvn)
                v_in = dramp.tile([128, T // 128, 128], BF16, tag="v_in")
                nc.sync.dma_start(out=v_in, in_=v_nat[:, :, :])
                v_out = dramp.tile([G, 128, T // 128, 128], BF16, tag="v_out")
                nc.gpsimd.collective_compute(
                    "AllGather", mybir.AluOpType.bypass,
                    replica_groups=GROUPS,
                    ins=[v_in.opt()], outs=[v_out.opt()],
                )

                # q projections (by M-chunk; chunk mc holds heads mc, mc+7)
                qT = attnp.tile([128, KC, T], BF16, tag="qT")
                for mc in range(KC):
                    wqc = wbig.tile([128, KC, 128], BF16, tag="wq")
                    nc.sync.dma_start(out=wqc, in_=wq_p[l, mc].rearrange("kc p m -> p kc m"))
                    proj_rope(wqc, mc * 128, qT[:, mc, :])

                for r in range(G):
                    nc.sync.dma_start(out=kT_pad[0:64, 0, r * T:(r + 1) * T],
                                      in_=k_out[r, 0:64, :])
                    nc.sync.dma_start(out=kT_pad[64:128, 1, r * T:(r + 1) * T],
                                      in_=k_out[r, 64:128, :])
                NTC = T // 128
                for r in range(G):
                    nc.sync.dma_start(out=vaug[:, r * NTC:(r + 1) * NTC, 0, 0:64],
                                      in_=v_out[r, :, :, 0:64])
                    nc.sync.dma_start(out=vaug[:, r * NTC:(r + 1) * NTC, 1, 64:128],
                                      in_=v_out[r, :, :, 64:128])

                # attention per head. Host permutes q-head order so chunk mc
                # holds heads (mc, mc+7): head h sits at partition base
                # (h//7)*64 == its kv-group's base in kT_full. Two key-chunks
                # share one 2-bank PSUM tile -> one Exp + one mask-mul per
                # pair. Denominators of all heads are collected into one PSUM
                # tile by one-hot matmuls (f32), inverted once, and broadcast
                # per chunk with selector matmuls.
                oT = attnp.tile([128, KC, T], BF16, tag="oT")
                aux = ps.tile([128, T], F32, tag="aux", bufs=1)
                for h in range(HQ):
                    mc, half = h % KC, h // KC
                    g = half
                    oacc = ps.tile([128, T], F32, tag="oacc", bufs=3)
                    for kp in range(NKP):
                        sp = ps.tile([128, 2, T], F32, tag="w", bufs=2)
                        for j in (0, 1):
                            kc = 2 * kp + j
                            nc.tensor.matmul(
                                sp[:, j, :],
                                kT_pad[:, half, kc * 128:(kc + 1) * 128],
                                qT[:, mc, :],
                                start=True, stop=True)
                        ex = expp.tile([128, 2, T], BF16, tag="expT")
                        nc.scalar.activation(out=ex, in_=sp, func=AF.Exp, scale=0.125)
                        nc.vector.tensor_mul(ex, ex, mask_sb[:, 2 * kp:2 * kp + 2, :])
                        for j in (0, 1):
                            kc = 2 * kp + j
                            nc.tensor.matmul(oacc, vaug[:, kc, half, :], ex[:, j, :],
                                             start=(kc == 0), stop=(kc == NKC - 1))
                    drow = 64 if half == 0 else 0
                    dsb = smallp.tile([128, T], BF16, tag="dsb")
                    nc.scalar.copy(dsb[drow:drow + 1, :], oacc[drow:drow + 1, :])
                    nc.tensor.matmul(aux[0:HQ, :], hot_sb[drow:drow + 1, h, :],
                                     dsb[drow:drow + 1, :],
                                     start=(h == 0), stop=(h == HQ - 1))
                    if half == 0:
                        nc.vector.tensor_copy(oT[0:64, mc, :], oacc[0:64, :])
                    else:
                        nc.vector.tensor_copy(oT[64:128, mc, :], oacc[64:128, :])

                rec = smallp.tile([HQ, T], F32, tag="rec")
                nc.vector.reciprocal_approx_fast(out=rec, in_=aux[0:HQ, :])
                recb = smallp.tile([HQ, T], BF16, tag="recb")
                nc.scalar.copy(recb, rec)

                # normalize oT chunk-by-chunk, o_proj + residual right behind
                for mc in range(KC):
                    rb = ps.tile([128, T], F32, tag="aux", bufs=1)
                    nc.tensor.matmul(rb, sel_sb[:, HQ + mc * 128:HQ + (mc + 1) * 128],
                                     recb, start=True, stop=True)
                    nc.vector.tensor_mul(oT[:, mc, :], oT[:, mc, :], rb)
                for mc in range(KC):
                    woc = wbig.tile([128, KC, 128], BF16, tag="wo")
                    nc.sync.dma_start(out=woc, in_=wo_p[l, mc].rearrange("kc p m -> p kc m"))
                    xd = ps.tile([128, T], F32, tag="w", bufs=2)
                    for k in range(KC):
                        nc.tensor.matmul(xd, woc[:, k, :], oT[:, k, :],
                                         start=(k == 0), stop=(k == KC - 1))
                    nc.vector.tensor_add(xT_sb[:, mc, :], xT_sb[:, mc, :], xd)

                # ---------------- MLP ----------------
                h2 = rmsnorm()
                m_sb = msbp.tile([128, MI, T], BF16, tag="m")
                for mi in range(MI):
                    wgc = wmlp.tile([128, KC, 128], BF16, tag="wg")
                    nc.sync.dma_start(out=wgc, in_=wg_p[l, mi].rearrange("kc p m -> p kc m"))
                    wuc = wmlp.tile([128, KC, 128], BF16, tag="wu")
                    nc.sync.dma_start(out=wuc, in_=wu_p[l, mi].rearrange("kc p m -> p kc m"))
                    pg = ps.tile([128, T], F32, tag="w", bufs=2)
                    for k in range(KC):
                        nc.tensor.matmul(pg, wgc[:, k, :], h2[:, k, :],
                                         start=(k == 0), stop=(k == KC - 1))
                    pu = ps.tile([128, T], F32, tag="w", bufs=2)
                    for k in range(KC):
                        nc.tensor.matmul(pu, wuc[:, k, :], h2[:, k, :],
                                         start=(k == 0), stop=(k == KC - 1))
                    sg = smallp.tile([128, T], BF16, tag="sg")
                    nc.scalar.activation(out=sg, in_=pg, func=AF.Silu)
                    nc.vector.tensor_mul(m_sb[:, mi, :], sg, pu)
                for mc in range(KC):
                    wdc = wmlp.tile([128, MI, 128], BF16, tag="wd")
                    nc.sync.dma_start(out=wdc, in_=wd_p[l, mc].rearrange("kci p m -> p kci m"))
                    xd = ps.tile([128, T], F32, tag="w", bufs=2)
                    for ki in range(MI):
                        nc.tensor.matmul(xd, wdc[:, ki, :], m_sb[:, ki, :],
                                         start=(ki == 0), stop=(ki == MI - 1))
                    nc.vector.tensor_add(xT_sb[:, mc, :], xT_sb[:, mc, :], xd)

            # final norm (lnf applied on host)
            ssq = ps.tile([1, T], F32, tag="w", bufs=2)
            for k in range(KC):
                sq = smallp.tile([128, T], BF16, tag="sq")
                nc.vector.tensor_mul(sq, xT_sb[:, k, :], xT_sb[:, k, :])
                nc.tensor.matmul(ssq, ones_col, sq, start=(k == 0), stop=(k == KC - 1))
            rmsv = smallp.tile([1, T], F32, tag="rmsv")
            nc.scalar.activation(out=rmsv, in_=ssq, func=AF.Sqrt, bias=eps_t, scale=1.0 / D)
            rstd = smallp.tile([1, T], F32, tag="rstd")
            nc.vector.reciprocal_approx_fast(out=rstd, in_=rmsv)
            rb = ps.tile([128, T], F32, tag="aux", bufs=1)
            nc.tensor.matmul(rb, ones_r128, rstd, start=True, stop=True)
            outT_r = outT_p.rearrange("(kc p) t -> p kc t", p=128)
            for k in range(KC):
                oc = smallp.tile([128, T], F32, tag="outc")
                nc.vector.tensor_mul(oc, xT_sb[:, k, :], rb)
                nc.sync.dma_start(out=outT_r[:, k, :], in_=oc)

    nc.finalize()
    return nc


def get_kernel(n_layers, with_bias):
    key = (n_layers, with_bias)
    if key not in _BUILD_CACHE:
        _BUILD_CACHE[key] = _build(n_layers, with_bias)
    return _BUILD_CACHE[key]


def _bf(a):
    return np.asarray(a, dtype=np.float32).astype(ml_dtypes.bfloat16)


def _rot_cols(w):
    """w @ R^T per 64-wide head block: out[:, :32] = -w[:, 32:64], out[:, 32:] = w[:, :32]."""
    nh = w.shape[1] // DH
    wr = w.reshape(w.shape[0], nh, DH)
    out = np.empty_like(wr)
    out[:, :, :DH // 2] = -wr[:, :, DH // 2:]
    out[:, :, DH // 2:] = wr[:, :, :DH // 2]
    return out.reshape(w.shape)


def _rot_vec(b):
    nh = b.shape[-1] // DH
    br = b.reshape(-1, nh, DH)
    out = np.empty_like(br)
    out[:, :, :DH // 2] = -br[:, :, DH // 2:]
    out[:, :, DH // 2:] = br[:, :, :DH // 2]
    return out.reshape(b.shape)


def _pack_mk(w):
    """[Din, Dout] -> [mc, kc, p, m] chunks for lhsT streaming."""
    din, dout = w.shape
    return np.ascontiguousarray(
        w.reshape(din // 128, 128, dout // 128, 128).transpose(2, 0, 1, 3))


def prepare_in_maps(inputs, n_layers, with_bias):
    return _prepare(n_layers=n_layers, with_bias_override=with_bias, **inputs)[0]


def _prepare(inputs_embeds, token_type_ids, attention_mask,
             wq, bq, wk, bk, wv, bv, wo, wg, wu, wd, ln1, ln2, lnf,
             n_layers=None, with_bias_override=None):
    f32 = np.float32
    inputs_embeds = np.asarray(inputs_embeds, f32)
    token_type_ids = np.asarray(token_type_ids)
    attention_mask = np.asarray(attention_mask, f32)
    wq, bq, wk, bk = np.asarray(wq, f32), np.asarray(bq, f32), np.asarray(wk, f32), np.asarray(bk, f32)
    wv, bv, wo = np.asarray(wv, f32), np.asarray(bv, f32), np.asarray(wo, f32)
    wg, wu, wd = np.asarray(wg, f32), np.asarray(wu, f32), np.asarray(wd, f32)
    ln1, ln2, lnf = np.asarray(ln1, f32), np.asarray(ln2, f32), np.asarray(lnf, f32)

    if n_layers is None:
        n_layers = N_LAYERS_OVERRIDE if N_LAYERS_OVERRIDE is not None else L
    with_bias = bool(np.any(bq[:n_layers]) or np.any(bk[:n_layers]) or np.any(bv[:n_layers]))
    if with_bias_override is not None:
        with_bias = with_bias or with_bias_override

    # ---- per-layer weight packing (ln folded in; RoPE rotation folded in) ----
    # head permutation: q-chunk mc holds heads (mc, mc+7) so that each head's
    # partition half matches its GQA kv-group's rows in kT_full
    perm = [h for p in range(KC) for h in (p, p + KC)]
    inv_sl = np.array(perm)

    def _perm_qcols(w):                    # permute 64-wide head column blocks
        return np.ascontiguousarray(
            w.reshape(w.shape[0], HQ, DH)[:, inv_sl, :].reshape(w.shape[0], HQ * DH))

    def _perm_orows(w):                    # permute 64-wide head row blocks
        return np.ascontiguousarray(
            w.reshape(HQ, DH, w.shape[1])[inv_sl].reshape(HQ * DH, w.shape[1]))

    wq_eff = ln1[:, :, None] * wq          # [L, D, 896]
    wk_eff = ln1[:, :, None] * wk          # [L, D, 128]
    wv_eff = ln1[:, :, None] * wv
    wg_eff = ln2[:, :, None] * wg
    wu_eff = ln2[:, :, None] * wu

    wq_pack = np.stack([_pack_mk(_perm_qcols(wq_eff[l])) for l in range(n_layers)])
    wo_pack = np.stack([_pack_mk(_perm_orows(wo[l])) for l in range(n_layers)])
    wg_pack = np.stack([_pack_mk(wg_eff[l]) for l in range(n_layers)])
    wu_pack = np.stack([_pack_mk(wu_eff[l]) for l in range(n_layers)])
    wd_pack = np.stack([_pack_mk(wd[l]) for l in range(n_layers)])
    wk_arr = wk_eff[:n_layers]
    wv_arr = wv_eff[:n_layers]

    # block-diag rotate-half matrix (two 64-head blocks), as lhsT = R^T
    r64 = np.zeros((DH, DH), np.float32)
    r64[:DH // 2, DH // 2:] = -np.eye(DH // 2, dtype=np.float32)
    r64[DH // 2:, :DH // 2] = np.eye(DH // 2, dtype=np.float32)
    rot2 = np.zeros((128, 128), np.float32)
    rot2[:DH, :DH] = r64.T
    rot2[DH:, DH:] = r64.T

    # selector constants (f32): cols 0:HQ = identity(HQ) for the one-hot
    # denominator gather; cols HQ+mc*128 : HQ+(mc+1)*128 broadcast head mc
    # (partitions 0:64) and head mc+7 (partitions 64:128) for chunk mc.
    hotp = np.tile(np.eye(HQ, dtype=np.float32).reshape(1, HQ * HQ), (128, 1))
    selp = np.zeros((HQ, HQ + KC * 128), np.float32)
    selp[:, :HQ] = np.eye(HQ, dtype=np.float32)
    for mc in range(KC):
        selp[mc, HQ + mc * 128: HQ + mc * 128 + 64] = 1.0
        selp[mc + KC, HQ + mc * 128 + 64: HQ + (mc + 1) * 128] = 1.0

    base = {
        "wq": _bf(wq_pack), "wk": _bf(wk_arr), "wv": _bf(wv_arr),
        "wo": _bf(wo_pack), "wg": _bf(wg_pack), "wu": _bf(wu_pack), "wd": _bf(wd_pack),
        "rotm": _bf(rot2), "selp": _bf(selp), "hotp": _bf(hotp),
    }
    if with_bias:
        def _perm_b(b):
            return b.reshape(n_layers, HQ, DH)[:, inv_sl, :].reshape(n_layers, HQ * DH)
        bias_pack = np.concatenate(
            [_perm_b(bq[:n_layers]), bk[:n_layers], bv[:n_layers]], axis=1)
        base["biasp"] = _bf(bias_pack)

    # ---- RoPE tables ----
    inv_freq = 1.0 / (THETA ** (np.arange(0, DH, 2, dtype=f32) / DH))
    ang = np.arange(S, dtype=f32)[:, None] * inv_freq[None, :]      # [S, 32]
    emb = np.concatenate([ang, ang], axis=-1)                        # [S, DH]
    cos_full, sin_full = np.cos(emb), np.sin(emb)                    # [S, DH]

    # ---- mask (multiplicative, per batch) ----
    t = token_type_ids
    tq = t[:, :, None]
    tk = t[:, None, :]
    qi = np.arange(S)[:, None]
    ki = np.arange(S)[None, :]
    allowed = ((tq == 0) & (tk == 0)) | ((tq == 1) & ((tk == 0) | ((tk == 1) & (ki <= qi))))
    m = allowed.astype(f32) * (attention_mask[:, None, :] > 0.5)     # [B, S(q), S(k)]

    in_maps = []
    for c in range(N_CORES):
        b, qt = c // G, c % G
        q0 = qt * T
        im = dict(base)
        im["xT"] = np.ascontiguousarray(inputs_embeds[b, q0:q0 + T, :].T)
        im["cosb"] = np.ascontiguousarray(np.tile(cos_full[q0:q0 + T].T, (2, 1)).astype(f32))
        im["sinb"] = np.ascontiguousarray(np.tile(sin_full[q0:q0 + T].T, (2, 1)).astype(f32))
        im["maskT"] = _bf(np.ascontiguousarray(m[b, q0:q0 + T, :].T))
        in_maps.append(im)

    global _LAST_IN_MAPS
    _LAST_IN_MAPS = in_maps
    return in_maps, n_layers, with_bias


def kernel(**inputs):
    in_maps, n_layers, with_bias = _prepare(**inputs)
    nc = get_kernel(n_layers, with_bias)
    res = run_bass_kernel_spmd(nc, in_maps, list(range(N_CORES)))
    lnf = np.asarray(inputs["lnf"], np.float32)
    out = np.empty((B, S, D), dtype=np.float32)
    for c in range(N_CORES):
        b, qt = c // G, c % G
        out[b, qt * T:(qt + 1) * T, :] = res.results[c]["outT"].T
    out *= lnf[None, None, :]
    return out
